# revision 18
# baseline (speedup 1.0000x reference)
"""Trainium2 Bass kernel for nn_BasicBlockShared (MoE-routed residual block).

Reference computation (per sample b):
    r = sigmoid(GAP(x) @ router_w.T + router_b)          # [B, E]
    k1 = sum_e r[b,e] * w1[e]                            # per-sample conv kernel
    y1 = relu(bn1(conv3x3(x[b], k1)))
    k2 = sum_e r[b,e] * w2[e]
    out = relu(bn2(conv3x3(y1, k2)) + x[b])

Sharding: data-parallel over batch. 32 samples -> 4 per core x 8 cores.
Expert banks + router + bn params replicated on every core.

v2: the expert combination runs on the TensorEngine instead of DVE/ScalarE.
Host side pre-casts the banks to bf16 (compute was already bf16 -> HBM
traffic halves) and re-lays them out per (layer, ig) as
    wb[ig, p=(e,o16), c*128 + i_loc]
where ocol=(og,dy,dx,o_loc) is split into 144 chunks c of 16 columns o16.
On chip, one matmul per chunk:
    psum[i_loc, (b,o16)] = wb_chunk[(e,o16), i_loc].T @ R[(e,o16), (b,o16')]
with R[(e,o16),(b,o16')] = r[e,b] * (o16==o16') -- a fixed 128x64
block-diagonal routing matrix built once from the router output (broadcast
DMA + mask multiply). PSUM accumulates in f32; ScalarE evicts 512-col tiles
to bf16 kT[ig] = [128 i, (b, 2304 ocol)] with a permuting access pattern.

Per-core plan:
  - x loaded into padded SBUF tiles xp[b][cg] = [128, 34, 34] bf16 (border
    memset to zero, interior cast in the SWDGE DMA).
  - Router: free-dim reduce for GAP, tiny fp32 matmul over channel groups,
    sigmoid+bias+1/HW scale on ScalarE, DRAM bounce to broadcast.
  - Conv: for each (b, og): two PSUM tiles [128 o, 512=(16 rows,32)] (row
    chunks) accumulate 18 shifted matmuls each; the weight tile is shared
    by the chunk pair (amortizes LDWEIGHTS, lets matmuls pipeline).
  - Epilogues: conv1: Relu(psum*s1+h1) on ScalarE -> padded y1p bf16.
    conv2: (psum*s2 + x) on VectorE, then Relu(.+h2) on ScalarE -> DMA out.
"""

import numpy as np
from contextlib import ExitStack

from concourse import bacc, mybir, tile
import concourse.bass as bass
from concourse.bass_utils import run_bass_kernel_spmd

B, C, H, W, E = 32, 256, 32, 32, 8
NCORES = 8
BS = B // NCORES            # samples per core
NG = C // 128               # channel groups (2)
OCOLS = NG * 3 * 3 * 128    # 2304 kernel cols per ig: (og, dy, dx, o_loc)
NCH = OCOLS // 16           # 144 combination chunks of 16 o-cols
CK = 24                     # chunks per bank DMA tile
PAD = H + 2                 # 34
EPS = 1e-5
BF = mybir.dt.bfloat16
F32 = mybir.dt.float32

_BUILT = {}


def build():
    nc = bacc.Bacc("TRN2", target_bir_lowering=False, debug=False,
                   num_devices=NCORES)
    x_d = nc.dram_tensor("x", [BS, C, H, W], BF, kind="ExternalInput")
    cp_d = nc.dram_tensor("cpack", [128, 32], F32, kind="ExternalInput")
    rb_d = nc.dram_tensor("router_b", [E], F32, kind="ExternalInput")
    w_d = [nc.dram_tensor("w1t", [NG, 128, NCH * 128], BF,
                          kind="ExternalInput"),
           nc.dram_tensor("w2t", [NG, 128, NCH * 128], BF,
                          kind="ExternalInput")]
    m16_d = nc.dram_tensor("mask16", [128, 16], BF, kind="ExternalInput")
    irep_d = nc.dram_tensor("irep", [E, 128], F32, kind="ExternalInput")
    out_d = nc.dram_tensor("out", [BS, C, H, W], F32, kind="ExternalOutput")

    with tile.TileContext(nc) as tc, ExitStack() as ctx:
        const = ctx.enter_context(tc.tile_pool(name="const", bufs=1))
        xpool = ctx.enter_context(tc.tile_pool(name="xpool", bufs=1))
        kpool = ctx.enter_context(tc.tile_pool(name="kpool", bufs=1))
        wpool = ctx.enter_context(tc.tile_pool(name="wpool", bufs=6))
        opool = ctx.enter_context(tc.tile_pool(name="opool", bufs=3))
        cpsum = ctx.enter_context(tc.tile_pool(name="cpsum", bufs=3, space="PSUM"))
        mpsum = ctx.enter_context(tc.tile_pool(name="mpsum", bufs=3, space="PSUM"))
        rpsum = ctx.enter_context(tc.tile_pool(name="rpsum", bufs=1, space="PSUM"))

        # ---- packed constants: cols (li*4+{g,b,m,v})*NG+g -> bn, 16+g*E+e
        #      -> router_w ----
        cp_sb = const.tile([128, 32], F32, tag="cpack")
        nc.sync.dma_start(out=cp_sb, in_=bass.AP(tensor=cp_d, offset=0,
                                                 ap=[[32, 128], [1, 32]]))
        # bn scale/shift: s = g * rsqrt(v + eps); h = b - m*s  [128, NG]
        eps_sb = const.tile([128, 1], F32, tag="eps")
        nc.vector.memset(eps_sb, EPS)
        bn_sh = {}
        for li in range(2):
            g_sb = cp_sb[:, li * 8 + 0:li * 8 + 2]
            b_sb = cp_sb[:, li * 8 + 2:li * 8 + 4]
            m_sb = cp_sb[:, li * 8 + 4:li * 8 + 6]
            v_sb = cp_sb[:, li * 8 + 6:li * 8 + 8]
            sq = const.tile([128, NG], F32, tag=f"bn_sq{li}")
            nc.scalar.activation(out=sq, in_=v_sb,
                                 func=mybir.ActivationFunctionType.Sqrt,
                                 bias=eps_sb, scale=1.0)
            rs = const.tile([128, NG], F32, tag=f"bn_rs{li}")
            nc.vector.reciprocal(out=rs, in_=sq)
            s_sb = const.tile([128, NG], F32, tag=f"bn_s{li}")
            nc.vector.tensor_mul(s_sb, g_sb, rs)
            t_sb = const.tile([128, NG], F32, tag=f"bn_t{li}")
            nc.vector.tensor_mul(t_sb, m_sb, s_sb)
            h_sb = const.tile([128, NG], F32, tag=f"bn_h{li}")
            nc.vector.tensor_sub(h_sb, b_sb, t_sb)
            bn_sh[li] = (s_sb, h_sb)
        m16_sb = const.tile([128, 16], BF, tag="m16")
        nc.sync.dma_start(out=m16_sb, in_=bass.AP(tensor=m16_d, offset=0,
                                                  ap=[[16, 128], [1, 16]]))
        irep_sb = const.tile([E, 128], F32, tag="irep")
        nc.sync.dma_start(out=irep_sb, in_=bass.AP(tensor=irep_d, offset=0,
                                                   ap=[[128, E], [1, 128]]))

        # ---- x -> unpadded bf16 staging (4KB-contiguous descriptors),
        #      then DVE pad-copy into bordered tiles ----
        xs = [[xpool.tile([128, H * W], BF, tag=f"xs_{b}_{g}",
                          name=f"xs_{b}_{g}")
               for g in range(NG)] for b in range(BS)]
        xp = [[xpool.tile([128, PAD, PAD], BF, tag=f"xp_{b}_{g}",
                          name=f"xp_{b}_{g}")
               for g in range(NG)] for b in range(BS)]
        y1p = [[xpool.tile([128, PAD, PAD], BF, tag=f"y1p_{b}_{g}",
                           name=f"y1p_{b}_{g}")
                for g in range(NG)] for b in range(BS)]
        for b in range(BS):
            for g in range(NG):
                src = bass.AP(tensor=x_d,
                              offset=(b * C + g * 128) * H * W,
                              ap=[[H * W, 128], [1, H * W]])
                nc.gpsimd.dma_start(out=xs[b][g], in_=src)

        # ---- bank streaming DMAs (order = consumption order) ----
        # per (l, half, ig): 72 chunks = 3 DMA tiles of CK=24 chunks.
        wt = {}
        for li in range(2):
            for half in range(2):
                for ig in range(NG):
                    for j in range(3):
                        t = wpool.tile([128, CK * 128], BF, tag="wt",
                                       name=f"wt_{li}_{half}_{ig}_{j}")
                        c0 = (half * 72 + j * CK) * 128
                        nc.gpsimd.dma_start(
                            out=t,
                            in_=bass.AP(tensor=w_d[li],
                                        offset=ig * 128 * NCH * 128 + c0,
                                        ap=[[NCH * 128, 128], [1, CK * 128]]))
                        wt[(li, half, ig, j)] = t

        for b in range(BS):
            for g in range(NG):
                for t in (xp[b][g], y1p[b][g]):
                    nc.gpsimd.memset(t[:, 0:PAD:33, :], 0.0)   # rows 0, 33
                    nc.gpsimd.memset(t[:, 1:33, 0:PAD:33], 0.0)  # cols 0, 33

        # ---- router ----
        gap = [const.tile([128, BS], F32, tag=f"gap_{g}", name=f"gap_{g}")
               for g in range(NG)]
        for b in range(BS):
            for g in range(NG):
                nc.vector.tensor_reduce(out=gap[g][:, b:b + 1],
                                        in_=xs[b][g].rearrange(
                                            "p (r c) -> p r c", r=H),
                                        axis=mybir.AxisListType.XY,
                                        op=mybir.AluOpType.add)
        rb_sb = const.tile([E, 1], F32, tag="rb")
        nc.sync.dma_start(out=rb_sb,
                          in_=bass.AP(tensor=rb_d, offset=0,
                                      ap=[[1, E], [1, 1]]))
        ps_r = rpsum.tile([E, BS], F32, tag="rps", name="ps_r")
        for g in range(NG):
            nc.tensor.matmul(ps_r, cp_sb[:, 16 + g * E:16 + (g + 1) * E],
                             gap[g], start=(g == 0), stop=(g == NG - 1))
        r_sb = const.tile([E, BS], F32, tag="r_sb")
        nc.scalar.activation(out=r_sb, in_=ps_r,
                             func=mybir.ActivationFunctionType.Sigmoid,
                             bias=rb_sb, scale=1.0 / (H * W))
        # re_bc[p, b] = r[p // 16, b] via PE broadcast (irep[e,p] = p//16==e)
        re_bc = rpsum.tile([128, BS], F32, tag="re_bc", name="re_bc")
        nc.tensor.matmul(re_bc, irep_sb, r_sb, start=True, stop=True)
        # R[(e,o16), (b,o16')] = r[e,b] * (o16 == o16')
        r_mat = const.tile([128, BS * 16], BF, tag="r_mat")
        for b in range(BS):
            nc.vector.tensor_scalar_mul(r_mat[:, b * 16:(b + 1) * 16],
                                        m16_sb, re_bc[:, b:b + 1])
        # pad-copies after r_mat on DVE (they only gate conv, not comb)
        for b in range(BS):
            for g in range(NG):
                nc.vector.tensor_scalar_mul(
                    xp[b][g][:, 1:33, 1:33],
                    xs[b][g].rearrange("p (r c) -> p r c", r=H), 1.0)

        # ---- combination for one og-half of one layer ----
        kT = [[kpool.tile([128, BS, OCOLS], BF, tag=f"kT_{li}_{ig}",
                          name=f"kT_{li}_{ig}")
               for ig in range(NG)] for li in range(2)]

        def comb_half(li, half):
            for ig in range(NG):
                for j in range(3):
                    t = wt[(li, half, ig, j)]
                    for grp in range(CK // 8):
                        mps = mpsum.tile([128, 512], F32, tag="mps",
                                         name=f"mps_{li}_{half}_{ig}_{j}_{grp}")
                        for cc in range(8):
                            col = (grp * 8 + cc) * 128
                            nc.tensor.matmul(
                                mps[:, cc * 64:(cc + 1) * 64],
                                t[:, col:col + 128], r_mat,
                                start=True, stop=True)
                        # evict: psum (cc, b, o16) -> kT (b, cc*16+o16)
                        src = mps.rearrange("p (cc bb o) -> p bb cc o",
                                            cc=8, bb=BS, o=16)
                        d0 = (half * 72 + j * CK + grp * 8) * 16
                        dst = kT[li][ig][:, :, d0:d0 + 128].rearrange(
                            "p b (cc o) -> p b cc o", cc=8, o=16)
                        nc.scalar.activation(
                            out=dst, in_=src,
                            func=mybir.ActivationFunctionType.Copy,
                            bias=0.0, scale=1.0)

        # ---- conv + epilogue for one (layer, og) ----
        def conv_og(li, og):
            src_t = xp if li == 0 else y1p
            s_sb, h_sb = bn_sh[li]
            for b in range(BS):
                pst = [cpsum.tile([128, 512], F32, tag="cps",
                                  name=f"cps_{li}_{og}_{b}_{c}")
                       for c in range(2)]
                for ig in range(NG):
                    for dy in range(3):
                        for dx in range(3):
                            t = ig * 9 + dy * 3 + dx
                            lhsT = kT[li][ig][
                                :, b,
                                og * 1152 + (dy * 3 + dx) * 128:
                                og * 1152 + (dy * 3 + dx) * 128 + 128]
                            for c in range(2):
                                nc.tensor.matmul(
                                    pst[c], lhsT,
                                    src_t[b][ig][:, c * 16 + dy:c * 16 + dy + 16,
                                                 dx:dx + 32],
                                    start=(t == 0), stop=(t == 17))
                for c in range(2):
                    ps = pst[c]
                    psr = ps.rearrange("p (r c) -> p r c", r=16)
                    if li == 0:
                        nc.scalar.activation(
                            out=y1p[b][og][:, 1 + c * 16:1 + c * 16 + 16,
                                           1:33],
                            in_=psr,
                            func=mybir.ActivationFunctionType.Relu,
                            bias=h_sb[:, og:og + 1],
                            scale=s_sb[:, og:og + 1])
                    else:
                        nc.vector.scalar_tensor_tensor(
                            out=psr, in0=psr, scalar=s_sb[:, og:og + 1],
                            in1=xp[b][og][:, 1 + c * 16:1 + c * 16 + 16,
                                          1:33],
                            op0=mybir.AluOpType.mult,
                            op1=mybir.AluOpType.add)
                        osb = opool.tile([128, 16, 32], F32, tag="osb")
                        nc.scalar.activation(
                            out=osb, in_=psr,
                            func=mybir.ActivationFunctionType.Relu,
                            bias=h_sb[:, og:og + 1], scale=1.0)
                        dst = bass.AP(
                            tensor=out_d,
                            offset=(b * C + og * 128) * H * W + c * 16 * W,
                            ap=[[H * W, 128], [1, 16 * W]])
                        nc.sync.dma_start(
                            out=dst,
                            in_=osb.rearrange("p r c -> p (r c)"))

        # ---- schedule: comb and conv alternate on the PE ----
        comb_half(0, 0)
        conv_og(0, 0)
        comb_half(0, 1)
        conv_og(0, 1)
        comb_half(1, 0)
        comb_half(1, 1)
        conv_og(1, 0)
        conv_og(1, 1)
    nc.compile()
    return nc


def _get_nc():
    if "nc" not in _BUILT:
        _BUILT["nc"] = build()
    return _BUILT["nc"]


def _host_transform_bank(w):
    """[E, O, I, 3, 3] f32 -> [ig, p=(e,o16), c*128+i_loc] bf16."""
    import ml_dtypes
    wr = w.reshape(E, NG, 128, NG, 128, 3, 3)     # e og o_l ig i_l dy dx
    t = wr.transpose(3, 0, 1, 5, 6, 2, 4)          # ig e og dy dx o_l i_l
    t = t.reshape(NG, E, OCOLS, 128)               # ig e ocol i
    t = t.reshape(NG, E, NCH, 16, 128)             # ig e c o16 i
    t = t.transpose(0, 1, 3, 2, 4)                 # ig e o16 c i
    t = t.reshape(NG, 128, NCH * 128)
    return np.ascontiguousarray(t.astype(ml_dtypes.bfloat16))


def _host_mask16():
    import ml_dtypes
    m = (np.arange(128)[:, None] % 16 == np.arange(16)[None, :])
    return m.astype(ml_dtypes.bfloat16)


def _host_irep():
    return (np.arange(128)[None, :] // 16 ==
            np.arange(E)[:, None]).astype(np.float32)


def _host_cpack(f):
    """[128, 32] f32: cols (li*4+{g,b,m,v})*NG+g = bn, 16+g*E+e = router_w."""
    cp = np.zeros((128, 32), np.float32)
    for li, names in enumerate((("g1", "b1", "m1", "v1"),
                                ("g2", "b2", "m2", "v2"))):
        for k, nm in enumerate(names):
            cp[:, li * 8 + k * 2:li * 8 + k * 2 + 2] = \
                f[nm].reshape(NG, 128).T
    for g in range(NG):
        cp[:, 16 + g * E:16 + (g + 1) * E] = \
            f["router_w"][:, g * 128:(g + 1) * 128].T
    return cp


def run(inputs, trace=False):
    import ml_dtypes
    nc = _get_nc()
    full = {k: np.ascontiguousarray(np.asarray(v, dtype=np.float32))
            for k, v in inputs.items()}
    full["w1t"] = _host_transform_bank(full.pop("w1"))
    full["w2t"] = _host_transform_bank(full.pop("w2"))
    full["mask16"] = _host_mask16()
    full["irep"] = _host_irep()
    full["cpack"] = _host_cpack(full)
    for k in ("g1", "b1", "m1", "v1", "g2", "b2", "m2", "v2", "router_w"):
        full.pop(k)
    xbf = np.ascontiguousarray(full.pop("x").astype(ml_dtypes.bfloat16))
    in_maps = []
    for j in range(NCORES):
        m = dict(full)
        m["x"] = np.ascontiguousarray(xbf[j * BS:(j + 1) * BS])
        in_maps.append(m)
    res = run_bass_kernel_spmd(nc, in_maps, core_ids=list(range(NCORES)),
                               trace=trace)
    out = np.concatenate([res.results[j]["out"] for j in range(NCORES)],
                         axis=0)
    return out, res


def kernel(**inputs) -> np.ndarray:
    out, _ = run(inputs, trace=False)
    return out
